# revision 14
# baseline (speedup 1.0000x reference)
"""Segment-mean kernel for nn_AttentionedSumLayer (Trainium2, 8 NeuronCores).

The reference's score chain is dead code (exp scores are overwritten with
ones), so the computation reduces to a segment mean over token rows:
    out[n, :] = mean(data[i, :] for i with tokens_to_node_map[i] == n)
with out[n] = 0 for empty nodes.  data is (1M, 256) f32, 100k nodes.

Strategy (memory-bound; minimize HBM bytes AND engine work):
  * Tokens of nodes with count >= 5 stream as fp8 E3M4 (1 B/el), produced
    with per-segment-column error feedback so the device-side segment sum
    carries only the final residual (~2^-5 of one element / count).
    Small nodes (1 <= count <= 4) go in a tiny bf16 side stream (~1% of
    tokens) to keep their worst-case error at bf16 level.
  * Main-node layout: nodes sorted by count desc, dealt round-robin to the
    8 cores (identical count profile per core -> one static SPMD program).
    Each window holds 128 nodes, one per PSUM partition; a node's tokens
    lie along the free dim of its own partition.  Tile k of a window is
    [128, 256] = token k of every node, so the PE accumulates tiles with a
    CONSTANT identity stationary operand -- no per-tile one-hot build at
    all.  Count-sorted order means a window mixes at most neighboring
    counts; missing slots ship as zero rows (~2% padding).
  * The PE alone would be the bottleneck (~110us vs ~105us of DMA), so
    ~1/6 of windows go to the otherwise-idle DVE as free-dim tensor_reduce
    over an F-major layout, with their own input stream DMA'd from the
    vector queue so they never stall the PE pipeline.
  * PSUM/accumulators are scaled by per-partition 1/count on ACT and
    streamed out as fp16.  Output DMAs ride the scalar queue so a waiting
    store never blocks input chunk loads (sync queue is FIFO).
"""

import math
import os

import numpy as np

NUM_NODES = 100000
N_CORES = 8
P = 128
F = 256
C_FB = 5  # nodes with 0 < count < C_FB use the bf16 fallback stream
DVE_EVERY = 6  # every DVE_EVERY-th window reduces on DVE instead of PE

TRACE = os.environ.get("BASS_PROBLEM_TRACE", "") == "1"
LAST_RESULTS = None  # BassKernelResults of the last run (for test.py)


# ---------------------------------------------------------------------------
# workaround: this walrus build rejects instructions carrying more than one
# sem wait ("Too many sync wait commands", CoreV*GenImpl setupSyncWait).
# After Tile scheduling, hoist excess waits onto same-engine NoOps inserted
# immediately before the over-limit instruction (waits only delay, so moving
# them earlier on the same engine is sound).
_MAX_WAITS = 1


def _split_waits(nc):
    import concourse.mybir as mybir

    uid = 0
    for f in nc.m.functions:
        for bb in f.blocks:
            out = []
            for inst in bb.instructions:
                si = inst.sync_info
                if si is not None and len(si.on_wait) > _MAX_WAITS:
                    waits = list(si.on_wait)
                    extra, keep = waits[:-_MAX_WAITS], waits[-_MAX_WAITS:]
                    for i in range(0, len(extra), _MAX_WAITS):
                        nop = mybir.InstNoOp(
                            name=f"wsplit-{uid}", engine=inst.engine
                        )
                        uid += 1
                        nop.sync_info = mybir.SyncInfo(
                            on_wait=extra[i : i + _MAX_WAITS], on_update=[]
                        )
                        out.append(nop)
                    si.on_wait = keep
                out.append(inst)
            bb.instructions = out


# ---------------------------------------------------------------------------
def _enable_profiling():
    """Best-effort: register the axon NTFF profile hook shim so trace=True
    works (antenv.axon_hooks is absent in this image) and stub the fish
    artifact upload.  Returns True when profiling is available."""
    try:
        import sys, types

        from trn_agent_boot.trn_boot import _ntff_profile_via_ctypes
        from concourse import bass_utils

        if "antenv.axon_hooks" not in sys.modules:
            hook = _ntff_profile_via_ctypes("/opt/axon/libaxon_pjrt.so")
            if hook is None:
                return False
            mod = types.ModuleType("antenv.axon_hooks")
            mod.get_axon_ntff_profile_hook = lambda: hook
            sys.modules["antenv.axon_hooks"] = mod
        bass_utils.upload_artifacts = lambda tmpdir: f"local://{tmpdir}"
        return True
    except Exception:
        return False


# ---------------------------------------------------------------------------
def _ef_quantize(d_sorted, seg_ids, qdt):
    """Error-feedback quantize rows (sorted by segment) to qdt: each row is
    quantized together with the residual carried from the previous row of
    the same segment, so the segment sum of quantized rows differs from the
    exact sum only by the final residual."""
    n, ncol = d_sorted.shape
    q = np.empty((n, ncol), qdt)
    if n == 0:
        return q
    starts = np.r_[0, np.flatnonzero(np.diff(seg_ids)) + 1]
    lens = np.diff(np.r_[starts, n])
    rank = np.arange(n) - np.repeat(starts, lens)
    maxr = int(rank.max())
    idx_by_rank = [np.flatnonzero(rank == r) for r in range(maxr + 1)]
    CB = 64  # column block to bound residual memory
    for c0 in range(0, ncol, CB):
        blk = d_sorted[:, c0 : c0 + CB]
        resid = np.zeros((n, blk.shape[1]), np.float32)
        for r in range(maxr + 1):
            idx = idx_by_rank[r]
            x = blk[idx]
            if r:
                x = x + resid[idx - 1]
            qx = x.astype(qdt)
            q[idx, c0 : c0 + CB] = qx
            if r < maxr:
                resid[idx] = x - qx.astype(np.float32)
    return q


def _grouped_arange(cnts):
    """[0..c0), [0..c1), ... concatenated."""
    total = int(cnts.sum())
    ends = np.cumsum(cnts)
    return np.arange(total) - np.repeat(ends - cnts, cnts)


# ---------------------------------------------------------------------------
def _preprocess(data, tokens_map):
    """Sort/arrange full inputs into per-core SPMD-uniform streams."""
    import ml_dtypes

    bf16 = ml_dtypes.bfloat16
    e3m4 = ml_dtypes.float8_e3m4

    m = np.asarray(tokens_map).astype(np.int64).ravel()
    data = np.ascontiguousarray(np.asarray(data, dtype=np.float32))

    counts = np.bincount(m, minlength=NUM_NODES)
    inv = np.zeros(NUM_NODES, np.float32)
    nz = counts > 0
    inv[nz] = 1.0 / counts[nz]

    order = np.argsort(m, kind="stable")
    sorted_nodes = m[order]

    # ---- main stream: tokens of count >= C_FB nodes, e3m4 + error feedback
    main_sel = counts[sorted_nodes] >= C_FB
    main_toks = order[main_sel]
    main_nodes = sorted_nodes[main_sel]  # sorted by node id, with repeats
    qmain = _ef_quantize(data[main_toks], main_nodes, e3m4)

    main_ids = np.unique(main_nodes)
    # count-desc global order, stable by id; pad to a multiple of 8 with -1
    byc = np.argsort(-counts[main_ids], kind="stable")
    glist = main_ids[byc]
    pad8 = (-len(glist)) % N_CORES
    glist = np.r_[glist, np.full(pad8, -1, np.int64)]
    n_rows = len(glist) // N_CORES  # nodes per core
    n_win = math.ceil(n_rows / P)
    padw = n_win * P - n_rows
    core_nodes = [
        np.r_[glist[c::N_CORES], np.full(padw, -1, np.int64)]
        for c in range(N_CORES)
    ]
    cnt_mat = np.stack(
        [np.where(cn >= 0, counts[np.maximum(cn, 0)], 0) for cn in core_nodes]
    )  # (N_CORES, n_win*P)
    # per-window tile counts: max count inside the window across all cores
    S = cnt_mat.reshape(N_CORES, n_win, P).max(axis=(0, 2)).astype(np.int64)
    S = np.maximum(S, 1)

    # split windows: every DVE_EVERY-th reduces on DVE (F-major layout,
    # separate stream); the rest accumulate on the PE (token-major).
    dve_win = (np.arange(n_win) % DVE_EVERY) == (DVE_EVERY - 1)
    pe_list = np.flatnonzero(~dve_win)
    dve_list = np.flatnonzero(dve_win)
    n_pe, n_dve = len(pe_list), len(dve_list)
    # window -> output slot (PE windows first, then DVE windows)
    wslot = np.empty(n_win, np.int64)
    wslot[pe_list] = np.arange(n_pe)
    wslot[dve_list] = n_pe + np.arange(n_dve)
    # stream-local tile offsets
    sidx = np.empty(n_win, np.int64)  # index within own stream
    sidx[pe_list] = np.arange(n_pe)
    sidx[dve_list] = np.arange(n_dve)
    S_pe = S[pe_list]
    S_dve = S[dve_list]
    offS_pe = np.r_[0, np.cumsum(S_pe)]
    offS_dve = np.r_[0, np.cumsum(S_dve)]
    T_pe = int(offS_pe[-1])
    T_dve = int(offS_dve[-1]) if n_dve else 1

    def node_row_start(ids):
        return np.searchsorted(main_nodes, ids)

    # ---- fallback: nodes with 1 <= count < C_FB, bf16, dealt round-robin
    fb_ids = np.flatnonzero((counts > 0) & (counts < C_FB))
    fb_core = [fb_ids[c::N_CORES] for c in range(N_CORES)]
    n_fb = max(len(x) for x in fb_core)
    n_fb_pos = math.ceil(n_fb / P)
    fb_tok_core = []
    for c in range(N_CORES):
        ids = fb_core[c]
        toks = []
        for w in range(n_fb_pos):
            win = ids[w * P : (w + 1) * P]
            tw, rw = [], []
            for k, node in enumerate(win):
                lo, hi = np.searchsorted(sorted_nodes, [node, node + 1])
                tt = order[lo:hi]
                tw.extend(tt.tolist())
                rw.extend([k] * len(tt))
            toks.append((np.array(tw, np.int64), np.array(rw, np.int64)))
        fb_tok_core.append(toks)
    S_fb = np.zeros(max(n_fb_pos, 1), np.int64)
    for w in range(n_fb_pos):
        S_fb[w] = max(
            1, max(-(-len(fb_tok_core[c][w][0]) // P) for c in range(N_CORES))
        )
    T_fb = int(S_fb.sum()) if n_fb_pos else 1

    in_maps = []
    for c in range(N_CORES):
        cn = core_nodes[c]
        cnts = cnt_mat[c]
        idx = np.flatnonzero(cn >= 0)
        ncnt = cnts[idx]
        src = np.repeat(node_row_start(cn[idx]), ncnt) + _grouped_arange(ncnt)
        prow = np.repeat(idx % P, ncnt)
        wtok = np.repeat(idx // P, ncnt)
        ktok = _grouped_arange(ncnt)
        on_dve = dve_win[wtok]

        pe = ~on_dve
        buf = np.zeros((P, T_pe, F), e3m4)
        buf[prow[pe], offS_pe[sidx[wtok[pe]]] + ktok[pe]] = qmain[src[pe]]

        vbuf = np.zeros((P, T_dve * F), e3m4)
        dv = np.flatnonzero(on_dve)
        if len(dv):
            Sw_tok = S[wtok[dv]]
            colbase = offS_dve[sidx[wtok[dv]]] * F + ktok[dv]
            cols = colbase[:, None] + Sw_tok[:, None] * np.arange(F)[None, :]
            vbuf[prow[dv][:, None], cols] = qmain[src[dv]]

        invm = np.zeros((P, n_win), np.float32)
        invm[idx % P, wslot[idx // P]] = inv[cn[idx]]

        fbstream = np.zeros((P, T_fb * F), bf16)
        fbrel = np.full((P, T_fb), -1.0, bf16)
        fbinv = np.zeros((P, max(n_fb_pos, 1)), np.float32)
        t0 = 0
        for w in range(n_fb_pos):
            Sw = int(S_fb[w])
            tw, rw = fb_tok_core[c][w]
            n = len(tw)
            L = P * Sw
            blk = np.zeros((L, F), bf16)
            if n:
                blk[:n] = data[tw].astype(bf16)
            fbstream[:, t0 * F : (t0 + Sw) * F] = blk.reshape(P, Sw * F)
            relblk = np.full(L, -1.0, bf16)
            relblk[:n] = rw.astype(bf16)
            fbrel[:, t0 : t0 + Sw] = relblk.reshape(P, Sw)
            win = fb_core[c][w * P : (w + 1) * P]
            fbinv[: len(win), w] = inv[win]
            t0 += Sw

        in_maps.append(
            {
                "data": buf.reshape(P, T_pe * F),
                "dvedata": vbuf,
                "ident": np.eye(P, dtype=e3m4),
                "invc": invm,
                "fbdata": fbstream,
                "fbrel": fbrel,
                "fbinv": fbinv,
            }
        )

    meta = {
        "S_pe": S_pe,
        "S_dve": S_dve,
        "offS_dve": offS_dve,
        "n_win": n_win,
        "n_pe": n_pe,
        "n_dve": n_dve,
        "T_pe": T_pe,
        "T_dve": T_dve,
        "wslot": wslot,
        "core_nodes": core_nodes,
        "S_fb": S_fb,
        "n_fb_pos": n_fb_pos,
        "T_fb": T_fb,
        "fb_core": fb_core,
    }
    return in_maps, meta


# ---------------------------------------------------------------------------
def _build_kernel(meta):
    import concourse.bass as bass
    import concourse.mybir as mybir
    from concourse.tile import TileContext

    f32 = mybir.dt.float32
    bf16 = mybir.dt.bfloat16
    fp8 = mybir.dt.float8e3
    f16 = mybir.dt.float16

    S_pe, S_dve, offS_dve = meta["S_pe"], meta["S_dve"], meta["offS_dve"]
    n_win, n_pe, n_dve = meta["n_win"], meta["n_pe"], meta["n_dve"]
    T_pe, T_dve = meta["T_pe"], meta["T_dve"]
    S_fb, n_fb_pos, T_fb = meta["S_fb"], meta["n_fb_pos"], meta["T_fb"]

    nc = bass.Bass()
    data_d = nc.dram_tensor("data", (P, T_pe * F), fp8, kind="ExternalInput")
    dve_d = nc.dram_tensor("dvedata", (P, T_dve * F), fp8, kind="ExternalInput")
    ident_d = nc.dram_tensor("ident", (P, P), fp8, kind="ExternalInput")
    inv_d = nc.dram_tensor("invc", (P, n_win), f32, kind="ExternalInput")
    fbdata_d = nc.dram_tensor("fbdata", (P, T_fb * F), bf16, kind="ExternalInput")
    fbrel_d = nc.dram_tensor("fbrel", (P, T_fb), bf16, kind="ExternalInput")
    fbinv_d = nc.dram_tensor(
        "fbinv", (P, max(n_fb_pos, 1)), f32, kind="ExternalInput"
    )
    out_d = nc.dram_tensor("out", (P, n_win * F), f16, kind="ExternalOutput")
    outfb_d = nc.dram_tensor(
        "outfb", (P, max(n_fb_pos, 1) * F), f16, kind="ExternalOutput"
    )

    OUT_BATCH = 8  # PE windows per output DMA
    VOUT_BATCH = 4  # DVE windows per output DMA
    CHUNK_TILES = 64  # tiles per PE input DMA / SBUF chunk
    FIRST_TILES = 16  # small first chunk so the PE starts sooner

    # batch PE windows into chunks of <= CHUNK_TILES tiles
    batches = []
    cur = []
    tiles = 0
    lim = FIRST_TILES
    for j in range(n_pe):
        if tiles + int(S_pe[j]) > lim and cur:
            batches.append(cur)
            cur, tiles = [], 0
            lim = CHUNK_TILES
        cur.append(j)
        tiles += int(S_pe[j])
    if cur:
        batches.append(cur)

    S_dve_max = int(max(S_dve)) if n_dve else 1

    with TileContext(nc) as tc:
        with (
            tc.tile_pool(name="const", bufs=1) as cpool,
            tc.tile_pool(name="chunk", bufs=3) as dpool,
            tc.tile_pool(name="vchunk", bufs=3) as vpool,
            tc.tile_pool(name="oh", bufs=2) as ohpool,
            tc.tile_pool(name="acc", bufs=2) as apool,
            tc.tile_pool(name="res", bufs=2) as rpool,
            tc.tile_pool(name="psum", bufs=4, space="PSUM") as ppool,
        ):
            ident_sb = cpool.tile([P, P], fp8)
            nc.sync.dma_start(ident_sb[:], ident_d[:])
            inv_sb = cpool.tile([P, n_win], f32)
            nc.sync.dma_start(inv_sb[:], inv_d[:])
            # first PE chunk starts loading before anything else queues
            jset0 = batches[0]
            Sb0 = int(sum(int(S_pe[j]) for j in jset0))
            chunk0 = dpool.tile([P, CHUNK_TILES * F], fp8, tag="chunk")
            nc.sync.dma_start(chunk0[:, : Sb0 * F], data_d[:, : Sb0 * F])

            fbrel_sb = cpool.tile([P, T_fb], bf16)
            nc.gpsimd.dma_start(fbrel_sb[:], fbrel_d[:])
            fbinv_sb = cpool.tile([P, max(n_fb_pos, 1)], f32)
            nc.gpsimd.dma_start(fbinv_sb[:], fbinv_d[:])
            iota_sb = cpool.tile([P, P], bf16)
            nc.gpsimd.iota(
                iota_sb[:],
                pattern=[[1, P]],
                base=0,
                channel_multiplier=0,
                allow_small_or_imprecise_dtypes=True,
            )

            # ---- fallback stream (bf16, tiny, DVE-built one-hots): fills
            # the pipeline-warmup bubble while main chunk 0 loads.
            if n_fb_pos:
                S_fb_max = int(max(S_fb))
                fchunk = vpool.tile([P, max(T_fb, S_dve_max) * F], bf16,
                                    tag="fchunk")
                nc.gpsimd.dma_start(fchunk[:, : T_fb * F], fbdata_d[:])
                fres = rpool.tile([P, n_fb_pos * F], f16, tag="fres")
                kb = 0
                for w in range(n_fb_pos):
                    Sw = int(S_fb[w])
                    oh = ohpool.tile([P, S_fb_max * P], bf16, tag="foh")
                    nc.vector.tensor_tensor(
                        out=oh[:, : Sw * P].rearrange("p (n f) -> p n f", f=P),
                        in0=iota_sb[:, None, :].to_broadcast([P, Sw, P]),
                        in1=fbrel_sb[:, kb : kb + Sw].to_broadcast([P, Sw, P]),
                        op=mybir.AluOpType.is_equal,
                    )
                    ps = ppool.tile([P, F], f32)
                    for k_ in range(Sw):
                        k = kb + k_
                        nc.tensor.matmul(
                            ps[:],
                            lhsT=oh[:, k_ * P : (k_ + 1) * P],
                            rhs=fchunk[:, k * F : (k + 1) * F],
                            start=(k_ == 0),
                            stop=(k_ == Sw - 1),
                        )
                    nc.scalar.activation(
                        fres[:, w * F : (w + 1) * F],
                        ps[:],
                        mybir.ActivationFunctionType.Copy,
                        scale=fbinv_sb[:, w : w + 1],
                    )
                    kb += Sw
                nc.scalar.dma_start(outfb_d[:], fres[:])

            # ---- main loops: PE batches with DVE windows interleaved.
            t0 = 0
            res = None
            vres = None
            vi = 0

            def do_dve_window(j):
                nonlocal vres
                Sw = int(S_dve[j])
                vchunk = vpool.tile([P, max(T_fb, S_dve_max) * F], fp8,
                                    tag="vchunk")
                nc.gpsimd.dma_start(
                    vchunk[:, : Sw * F],
                    dve_d[:, int(offS_dve[j]) * F : int(offS_dve[j] + Sw) * F],
                )
                acc = apool.tile([P, F], f32, tag="acc")
                nc.vector.tensor_reduce(
                    out=acc[:],
                    in_=vchunk[:, : Sw * F].rearrange("p (f k) -> p f k", k=Sw),
                    axis=mybir.AxisListType.X,
                    op=mybir.AluOpType.add,
                )
                slot = n_pe + j
                vb = j % VOUT_BATCH
                if vb == 0:
                    vres = rpool.tile([P, VOUT_BATCH * F], f16, tag="vres")
                nc.scalar.activation(
                    vres[:, vb * F : (vb + 1) * F],
                    acc[:],
                    mybir.ActivationFunctionType.Copy,
                    scale=inv_sb[:, slot : slot + 1],
                )
                if vb == VOUT_BATCH - 1 or j == n_dve - 1:
                    lo = (slot - vb) * F
                    nc.scalar.dma_start(
                        out_d[:, lo : (slot + 1) * F], vres[:, : (vb + 1) * F]
                    )

            for bi, jset in enumerate(batches):
                Sb = int(sum(int(S_pe[j]) for j in jset))
                if bi == 0:
                    chunk = chunk0
                else:
                    chunk = dpool.tile([P, CHUNK_TILES * F], fp8, tag="chunk")
                    nc.sync.dma_start(
                        chunk[:, : Sb * F], data_d[:, t0 * F : (t0 + Sb) * F]
                    )
                kb = 0
                for j in jset:
                    Sw = int(S_pe[j])
                    ps = ppool.tile([P, F], f32)
                    for k in range(Sw):
                        nc.tensor.matmul(
                            ps[:],
                            lhsT=ident_sb[:],
                            rhs=chunk[:, (kb + k) * F : (kb + k + 1) * F],
                            start=(k == 0),
                            stop=(k == Sw - 1),
                        )
                    jb = j % OUT_BATCH
                    if jb == 0:
                        res = rpool.tile([P, OUT_BATCH * F], f16, tag="res")
                    nc.scalar.activation(
                        res[:, jb * F : (jb + 1) * F],
                        ps[:],
                        mybir.ActivationFunctionType.Copy,
                        scale=inv_sb[:, j : j + 1],
                    )
                    if jb == OUT_BATCH - 1 or j == n_pe - 1:
                        lo = (j - jb) * F
                        nc.scalar.dma_start(
                            out_d[:, lo : (j + 1) * F], res[:, : (jb + 1) * F]
                        )
                    kb += Sw
                t0 += Sb
                target = round((bi + 1) * n_dve / len(batches))
                while vi < min(target, n_dve):
                    do_dve_window(vi)
                    vi += 1
            while vi < n_dve:
                do_dve_window(vi)
                vi += 1

    _split_waits(nc)
    return nc


# ---------------------------------------------------------------------------
def kernel(data, tokens_to_node_map, W=None, b=None, scoring=None):
    global LAST_RESULTS
    from concourse import bass_utils

    in_maps, meta = _preprocess(data, tokens_to_node_map)
    nc = _build_kernel(meta)

    kwargs = {}
    if TRACE and _enable_profiling():
        kwargs["trace"] = True
    res = None
    for attempt in range(3):
        try:
            res = bass_utils.run_bass_kernel_spmd(
                nc, in_maps, core_ids=list(range(N_CORES)), **kwargs
            )
            break
        except Exception:
            if attempt == 2:
                raise
            kwargs.pop("trace", None)  # drop profiling on retry
    LAST_RESULTS = res

    n_win = meta["n_win"]
    wslot = meta["wslot"]
    out = np.zeros((NUM_NODES, F), np.float32)
    for c in range(N_CORES):
        oc = res.results[c]["out"].astype(np.float32)
        oc = oc.reshape(P, n_win, F)
        cn = meta["core_nodes"][c]
        idx = np.flatnonzero(cn >= 0)
        out[cn[idx]] = oc[idx % P, wslot[idx // P]]
    for c in range(N_CORES):
        ids = meta["fb_core"][c]
        if not len(ids):
            continue
        ofb = res.results[c]["outfb"].astype(np.float32)
        for w in range(meta["n_fb_pos"]):
            win = ids[w * P : (w + 1) * P]
            out[win] = ofb[: len(win), w * F : (w + 1) * F]
    return out


# revision 15
# speedup vs baseline: 1.1228x; 1.1228x over previous
"""Segment-mean kernel for nn_AttentionedSumLayer (Trainium2, 8 NeuronCores).

The reference's score chain is dead code (exp scores are overwritten with
ones), so the computation reduces to a segment mean over token rows:
    out[n, :] = mean(data[i, :] for i with tokens_to_node_map[i] == n)
with out[n] = 0 for empty nodes.  data is (1M, 256) f32, 100k nodes.

Strategy (memory-bound; minimize HBM bytes AND engine work):
  * Tokens of nodes with count >= 5 stream as fp8 E3M4 (1 B/el), produced
    with per-segment-column error feedback so the device-side segment sum
    carries only the final residual (~2^-5 of one element / count).
    Small nodes (1 <= count <= 4) go in a tiny bf16 side stream (~1% of
    tokens) to keep their worst-case error at bf16 level.
  * Main-node layout: nodes sorted by count desc, dealt round-robin to the
    8 cores (identical count profile per core -> one static SPMD program).
    Each window holds 128 nodes, one per PSUM partition; a node's tokens
    lie along the free dim of its own partition.  Tile k of a window is
    [128, 256] = token k of every node, so the PE accumulates tiles with a
    CONSTANT identity stationary operand -- no per-tile one-hot build at
    all.  Count-sorted order means a window mixes at most neighboring
    counts; missing slots ship as zero rows (~2% padding).
  * The PE alone would be the bottleneck (~110us vs ~105us of DMA), so
    ~1/6 of windows go to the otherwise-idle DVE as free-dim tensor_reduce
    over an F-major layout, with their own input stream DMA'd from the
    vector queue so they never stall the PE pipeline.
  * PSUM/accumulators are scaled by per-partition 1/count on ACT and
    streamed out as fp16.  Output DMAs ride the scalar queue so a waiting
    store never blocks input chunk loads (sync queue is FIFO).
"""

import math
import os

import numpy as np

NUM_NODES = 100000
N_CORES = 8
P = 128
F = 256
C_FB = 5  # nodes with 0 < count < C_FB use the bf16 fallback stream
DVE_EVERY = 10**9  # every DVE_EVERY-th window reduces on DVE instead of PE (disabled)

TRACE = os.environ.get("BASS_PROBLEM_TRACE", "") == "1"
LAST_RESULTS = None  # BassKernelResults of the last run (for test.py)


# ---------------------------------------------------------------------------
# workaround: this walrus build rejects instructions carrying more than one
# sem wait ("Too many sync wait commands", CoreV*GenImpl setupSyncWait).
# After Tile scheduling, hoist excess waits onto same-engine NoOps inserted
# immediately before the over-limit instruction (waits only delay, so moving
# them earlier on the same engine is sound).
_MAX_WAITS = 1


def _split_waits(nc):
    import concourse.mybir as mybir

    uid = 0
    for f in nc.m.functions:
        for bb in f.blocks:
            out = []
            for inst in bb.instructions:
                si = inst.sync_info
                if si is not None and len(si.on_wait) > _MAX_WAITS:
                    waits = list(si.on_wait)
                    extra, keep = waits[:-_MAX_WAITS], waits[-_MAX_WAITS:]
                    for i in range(0, len(extra), _MAX_WAITS):
                        nop = mybir.InstNoOp(
                            name=f"wsplit-{uid}", engine=inst.engine
                        )
                        uid += 1
                        nop.sync_info = mybir.SyncInfo(
                            on_wait=extra[i : i + _MAX_WAITS], on_update=[]
                        )
                        out.append(nop)
                    si.on_wait = keep
                out.append(inst)
            bb.instructions = out


# ---------------------------------------------------------------------------
def _enable_profiling():
    """Best-effort: register the axon NTFF profile hook shim so trace=True
    works (antenv.axon_hooks is absent in this image) and stub the fish
    artifact upload.  Returns True when profiling is available."""
    try:
        import sys, types

        from trn_agent_boot.trn_boot import _ntff_profile_via_ctypes
        from concourse import bass_utils

        if "antenv.axon_hooks" not in sys.modules:
            hook = _ntff_profile_via_ctypes("/opt/axon/libaxon_pjrt.so")
            if hook is None:
                return False
            mod = types.ModuleType("antenv.axon_hooks")
            mod.get_axon_ntff_profile_hook = lambda: hook
            sys.modules["antenv.axon_hooks"] = mod
        bass_utils.upload_artifacts = lambda tmpdir: f"local://{tmpdir}"
        return True
    except Exception:
        return False


# ---------------------------------------------------------------------------
def _ef_quantize(d_sorted, seg_ids, qdt):
    """Error-feedback quantize rows (sorted by segment) to qdt: each row is
    quantized together with the residual carried from the previous row of
    the same segment, so the segment sum of quantized rows differs from the
    exact sum only by the final residual."""
    n, ncol = d_sorted.shape
    q = np.empty((n, ncol), qdt)
    if n == 0:
        return q
    starts = np.r_[0, np.flatnonzero(np.diff(seg_ids)) + 1]
    lens = np.diff(np.r_[starts, n])
    rank = np.arange(n) - np.repeat(starts, lens)
    maxr = int(rank.max())
    idx_by_rank = [np.flatnonzero(rank == r) for r in range(maxr + 1)]
    CB = 64  # column block to bound residual memory
    for c0 in range(0, ncol, CB):
        blk = d_sorted[:, c0 : c0 + CB]
        resid = np.zeros((n, blk.shape[1]), np.float32)
        for r in range(maxr + 1):
            idx = idx_by_rank[r]
            x = blk[idx]
            if r:
                x = x + resid[idx - 1]
            qx = x.astype(qdt)
            q[idx, c0 : c0 + CB] = qx
            if r < maxr:
                resid[idx] = x - qx.astype(np.float32)
    return q


def _grouped_arange(cnts):
    """[0..c0), [0..c1), ... concatenated."""
    total = int(cnts.sum())
    ends = np.cumsum(cnts)
    return np.arange(total) - np.repeat(ends - cnts, cnts)


# ---------------------------------------------------------------------------
def _preprocess(data, tokens_map):
    """Sort/arrange full inputs into per-core SPMD-uniform streams."""
    import ml_dtypes

    bf16 = ml_dtypes.bfloat16
    e3m4 = ml_dtypes.float8_e3m4

    m = np.asarray(tokens_map).astype(np.int64).ravel()
    data = np.ascontiguousarray(np.asarray(data, dtype=np.float32))

    counts = np.bincount(m, minlength=NUM_NODES)
    inv = np.zeros(NUM_NODES, np.float32)
    nz = counts > 0
    inv[nz] = 1.0 / counts[nz]

    order = np.argsort(m, kind="stable")
    sorted_nodes = m[order]

    # ---- main stream: tokens of count >= C_FB nodes, e3m4 + error feedback
    main_sel = counts[sorted_nodes] >= C_FB
    main_toks = order[main_sel]
    main_nodes = sorted_nodes[main_sel]  # sorted by node id, with repeats
    qmain = _ef_quantize(data[main_toks], main_nodes, e3m4)

    main_ids = np.unique(main_nodes)
    # count-desc global order, stable by id; pad to a multiple of 8 with -1
    byc = np.argsort(-counts[main_ids], kind="stable")
    glist = main_ids[byc]
    pad8 = (-len(glist)) % N_CORES
    glist = np.r_[glist, np.full(pad8, -1, np.int64)]
    n_rows = len(glist) // N_CORES  # nodes per core
    n_win = math.ceil(n_rows / P)
    padw = n_win * P - n_rows
    core_nodes = [
        np.r_[glist[c::N_CORES], np.full(padw, -1, np.int64)]
        for c in range(N_CORES)
    ]
    cnt_mat = np.stack(
        [np.where(cn >= 0, counts[np.maximum(cn, 0)], 0) for cn in core_nodes]
    )  # (N_CORES, n_win*P)
    # per-window tile counts: max count inside the window across all cores
    S = cnt_mat.reshape(N_CORES, n_win, P).max(axis=(0, 2)).astype(np.int64)
    S = np.maximum(S, 1)

    # split windows: every DVE_EVERY-th reduces on DVE (F-major layout,
    # separate stream); the rest accumulate on the PE (token-major).
    dve_win = (np.arange(n_win) % DVE_EVERY) == (DVE_EVERY - 1)
    pe_list = np.flatnonzero(~dve_win)
    dve_list = np.flatnonzero(dve_win)
    n_pe, n_dve = len(pe_list), len(dve_list)
    # window -> output slot (PE windows first, then DVE windows)
    wslot = np.empty(n_win, np.int64)
    wslot[pe_list] = np.arange(n_pe)
    wslot[dve_list] = n_pe + np.arange(n_dve)
    # stream-local tile offsets
    sidx = np.empty(n_win, np.int64)  # index within own stream
    sidx[pe_list] = np.arange(n_pe)
    sidx[dve_list] = np.arange(n_dve)
    S_pe = S[pe_list]
    S_dve = S[dve_list]
    offS_pe = np.r_[0, np.cumsum(S_pe)]
    offS_dve = np.r_[0, np.cumsum(S_dve)]
    T_pe = int(offS_pe[-1])
    T_dve = int(offS_dve[-1]) if n_dve else 1

    def node_row_start(ids):
        return np.searchsorted(main_nodes, ids)

    # ---- fallback: nodes with 1 <= count < C_FB, bf16, dealt round-robin
    fb_ids = np.flatnonzero((counts > 0) & (counts < C_FB))
    fb_core = [fb_ids[c::N_CORES] for c in range(N_CORES)]
    n_fb = max(len(x) for x in fb_core)
    n_fb_pos = math.ceil(n_fb / P)
    fb_tok_core = []
    for c in range(N_CORES):
        ids = fb_core[c]
        toks = []
        for w in range(n_fb_pos):
            win = ids[w * P : (w + 1) * P]
            tw, rw = [], []
            for k, node in enumerate(win):
                lo, hi = np.searchsorted(sorted_nodes, [node, node + 1])
                tt = order[lo:hi]
                tw.extend(tt.tolist())
                rw.extend([k] * len(tt))
            toks.append((np.array(tw, np.int64), np.array(rw, np.int64)))
        fb_tok_core.append(toks)
    S_fb = np.zeros(max(n_fb_pos, 1), np.int64)
    for w in range(n_fb_pos):
        S_fb[w] = max(
            1, max(-(-len(fb_tok_core[c][w][0]) // P) for c in range(N_CORES))
        )
    T_fb = int(S_fb.sum()) if n_fb_pos else 1

    in_maps = []
    for c in range(N_CORES):
        cn = core_nodes[c]
        cnts = cnt_mat[c]
        idx = np.flatnonzero(cn >= 0)
        ncnt = cnts[idx]
        src = np.repeat(node_row_start(cn[idx]), ncnt) + _grouped_arange(ncnt)
        prow = np.repeat(idx % P, ncnt)
        wtok = np.repeat(idx // P, ncnt)
        ktok = _grouped_arange(ncnt)
        on_dve = dve_win[wtok]

        pe = ~on_dve
        buf = np.zeros((P, T_pe, F), e3m4)
        buf[prow[pe], offS_pe[sidx[wtok[pe]]] + ktok[pe]] = qmain[src[pe]]

        vbuf = np.zeros((P, T_dve * F), e3m4)
        dv = np.flatnonzero(on_dve)
        if len(dv):
            Sw_tok = S[wtok[dv]]
            colbase = offS_dve[sidx[wtok[dv]]] * F + ktok[dv]
            cols = colbase[:, None] + Sw_tok[:, None] * np.arange(F)[None, :]
            vbuf[prow[dv][:, None], cols] = qmain[src[dv]]

        invm = np.zeros((P, n_win), np.float32)
        invm[idx % P, wslot[idx // P]] = inv[cn[idx]]

        fbstream = np.zeros((P, T_fb * F), bf16)
        fbrel = np.full((P, T_fb), -1.0, bf16)
        fbinv = np.zeros((P, max(n_fb_pos, 1)), np.float32)
        t0 = 0
        for w in range(n_fb_pos):
            Sw = int(S_fb[w])
            tw, rw = fb_tok_core[c][w]
            n = len(tw)
            L = P * Sw
            blk = np.zeros((L, F), bf16)
            if n:
                blk[:n] = data[tw].astype(bf16)
            fbstream[:, t0 * F : (t0 + Sw) * F] = blk.reshape(P, Sw * F)
            relblk = np.full(L, -1.0, bf16)
            relblk[:n] = rw.astype(bf16)
            fbrel[:, t0 : t0 + Sw] = relblk.reshape(P, Sw)
            win = fb_core[c][w * P : (w + 1) * P]
            fbinv[: len(win), w] = inv[win]
            t0 += Sw

        in_maps.append(
            {
                "data": buf.reshape(P, T_pe * F),
                "dvedata": vbuf,
                "ident": np.eye(P, dtype=e3m4),
                "invc": invm,
                "fbdata": fbstream,
                "fbrel": fbrel,
                "fbinv": fbinv,
            }
        )

    meta = {
        "S_pe": S_pe,
        "S_dve": S_dve,
        "offS_dve": offS_dve,
        "n_win": n_win,
        "n_pe": n_pe,
        "n_dve": n_dve,
        "T_pe": T_pe,
        "T_dve": T_dve,
        "wslot": wslot,
        "core_nodes": core_nodes,
        "S_fb": S_fb,
        "n_fb_pos": n_fb_pos,
        "T_fb": T_fb,
        "fb_core": fb_core,
    }
    return in_maps, meta


# ---------------------------------------------------------------------------
def _build_kernel(meta):
    import concourse.bass as bass
    import concourse.mybir as mybir
    from concourse.tile import TileContext

    f32 = mybir.dt.float32
    bf16 = mybir.dt.bfloat16
    fp8 = mybir.dt.float8e3
    f16 = mybir.dt.float16

    S_pe, S_dve, offS_dve = meta["S_pe"], meta["S_dve"], meta["offS_dve"]
    n_win, n_pe, n_dve = meta["n_win"], meta["n_pe"], meta["n_dve"]
    T_pe, T_dve = meta["T_pe"], meta["T_dve"]
    S_fb, n_fb_pos, T_fb = meta["S_fb"], meta["n_fb_pos"], meta["T_fb"]

    nc = bass.Bass()
    data_d = nc.dram_tensor("data", (P, T_pe * F), fp8, kind="ExternalInput")
    dve_d = nc.dram_tensor("dvedata", (P, T_dve * F), fp8, kind="ExternalInput")
    ident_d = nc.dram_tensor("ident", (P, P), fp8, kind="ExternalInput")
    inv_d = nc.dram_tensor("invc", (P, n_win), f32, kind="ExternalInput")
    fbdata_d = nc.dram_tensor("fbdata", (P, T_fb * F), bf16, kind="ExternalInput")
    fbrel_d = nc.dram_tensor("fbrel", (P, T_fb), bf16, kind="ExternalInput")
    fbinv_d = nc.dram_tensor(
        "fbinv", (P, max(n_fb_pos, 1)), f32, kind="ExternalInput"
    )
    out_d = nc.dram_tensor("out", (P, n_win * F), f16, kind="ExternalOutput")
    outfb_d = nc.dram_tensor(
        "outfb", (P, max(n_fb_pos, 1) * F), f16, kind="ExternalOutput"
    )

    OUT_BATCH = 8  # PE windows per output DMA
    VOUT_BATCH = 4  # DVE windows per output DMA
    CHUNK_TILES = 64  # tiles per PE input DMA / SBUF chunk
    FIRST_TILES = 16  # small first chunk so the PE starts sooner

    # batch PE windows into chunks of <= CHUNK_TILES tiles
    batches = []
    cur = []
    tiles = 0
    lim = FIRST_TILES
    for j in range(n_pe):
        if tiles + int(S_pe[j]) > lim and cur:
            batches.append(cur)
            cur, tiles = [], 0
            lim = CHUNK_TILES
        cur.append(j)
        tiles += int(S_pe[j])
    if cur:
        batches.append(cur)

    S_dve_max = int(max(S_dve)) if n_dve else 1

    with TileContext(nc) as tc:
        with (
            tc.tile_pool(name="const", bufs=1) as cpool,
            tc.tile_pool(name="chunk", bufs=3) as dpool,
            tc.tile_pool(name="vchunk", bufs=3) as vpool,
            tc.tile_pool(name="oh", bufs=2) as ohpool,
            tc.tile_pool(name="acc", bufs=2) as apool,
            tc.tile_pool(name="res", bufs=2) as rpool,
            tc.tile_pool(name="psum", bufs=4, space="PSUM") as ppool,
        ):
            ident_sb = cpool.tile([P, P], fp8)
            nc.sync.dma_start(ident_sb[:], ident_d[:])
            inv_sb = cpool.tile([P, n_win], f32)
            nc.sync.dma_start(inv_sb[:], inv_d[:])
            # first PE chunk starts loading before anything else queues
            jset0 = batches[0]
            Sb0 = int(sum(int(S_pe[j]) for j in jset0))
            chunk0 = dpool.tile([P, CHUNK_TILES * F], fp8, tag="chunk")
            nc.sync.dma_start(chunk0[:, : Sb0 * F], data_d[:, : Sb0 * F])

            fbrel_sb = cpool.tile([P, T_fb], bf16)
            nc.sync.dma_start(fbrel_sb[:], fbrel_d[:])
            fbinv_sb = cpool.tile([P, max(n_fb_pos, 1)], f32)
            nc.sync.dma_start(fbinv_sb[:], fbinv_d[:])
            iota_sb = cpool.tile([P, P], bf16)
            nc.gpsimd.iota(
                iota_sb[:],
                pattern=[[1, P]],
                base=0,
                channel_multiplier=0,
                allow_small_or_imprecise_dtypes=True,
            )

            # ---- fallback stream (bf16, tiny, DVE-built one-hots): fills
            # the pipeline-warmup bubble while main chunk 0 loads.
            if n_fb_pos:
                S_fb_max = int(max(S_fb))
                fchunk = vpool.tile([P, max(T_fb, S_dve_max) * F], bf16,
                                    tag="fchunk")
                nc.sync.dma_start(fchunk[:, : T_fb * F], fbdata_d[:])
                fres = rpool.tile([P, n_fb_pos * F], f16, tag="fres")
                kb = 0
                for w in range(n_fb_pos):
                    Sw = int(S_fb[w])
                    oh = ohpool.tile([P, S_fb_max * P], bf16, tag="foh")
                    nc.vector.tensor_tensor(
                        out=oh[:, : Sw * P].rearrange("p (n f) -> p n f", f=P),
                        in0=iota_sb[:, None, :].to_broadcast([P, Sw, P]),
                        in1=fbrel_sb[:, kb : kb + Sw].to_broadcast([P, Sw, P]),
                        op=mybir.AluOpType.is_equal,
                    )
                    ps = ppool.tile([P, F], f32)
                    for k_ in range(Sw):
                        k = kb + k_
                        nc.tensor.matmul(
                            ps[:],
                            lhsT=oh[:, k_ * P : (k_ + 1) * P],
                            rhs=fchunk[:, k * F : (k + 1) * F],
                            start=(k_ == 0),
                            stop=(k_ == Sw - 1),
                        )
                    nc.scalar.activation(
                        fres[:, w * F : (w + 1) * F],
                        ps[:],
                        mybir.ActivationFunctionType.Copy,
                        scale=fbinv_sb[:, w : w + 1],
                    )
                    kb += Sw
                nc.scalar.dma_start(outfb_d[:], fres[:])

            # ---- main loops: PE batches with DVE windows interleaved.
            t0 = 0
            res = None
            vres = None
            vi = 0

            def do_dve_window(j):
                nonlocal vres
                Sw = int(S_dve[j])
                vchunk = vpool.tile([P, max(T_fb, S_dve_max) * F], fp8,
                                    tag="vchunk")
                nc.gpsimd.dma_start(
                    vchunk[:, : Sw * F],
                    dve_d[:, int(offS_dve[j]) * F : int(offS_dve[j] + Sw) * F],
                )
                acc = apool.tile([P, F], f32, tag="acc")
                nc.vector.tensor_reduce(
                    out=acc[:],
                    in_=vchunk[:, : Sw * F].rearrange("p (f k) -> p f k", k=Sw),
                    axis=mybir.AxisListType.X,
                    op=mybir.AluOpType.add,
                )
                slot = n_pe + j
                vb = j % VOUT_BATCH
                if vb == 0:
                    vres = rpool.tile([P, VOUT_BATCH * F], f16, tag="vres")
                nc.scalar.activation(
                    vres[:, vb * F : (vb + 1) * F],
                    acc[:],
                    mybir.ActivationFunctionType.Copy,
                    scale=inv_sb[:, slot : slot + 1],
                )
                if vb == VOUT_BATCH - 1 or j == n_dve - 1:
                    lo = (slot - vb) * F
                    nc.scalar.dma_start(
                        out_d[:, lo : (slot + 1) * F], vres[:, : (vb + 1) * F]
                    )

            for bi, jset in enumerate(batches):
                Sb = int(sum(int(S_pe[j]) for j in jset))
                if bi == 0:
                    chunk = chunk0
                else:
                    chunk = dpool.tile([P, CHUNK_TILES * F], fp8, tag="chunk")
                    nc.sync.dma_start(
                        chunk[:, : Sb * F], data_d[:, t0 * F : (t0 + Sb) * F]
                    )
                kb = 0
                for j in jset:
                    Sw = int(S_pe[j])
                    ps = ppool.tile([P, F], f32)
                    for k in range(Sw):
                        nc.tensor.matmul(
                            ps[:],
                            lhsT=ident_sb[:],
                            rhs=chunk[:, (kb + k) * F : (kb + k + 1) * F],
                            start=(k == 0),
                            stop=(k == Sw - 1),
                        )
                    jb = j % OUT_BATCH
                    if jb == 0:
                        res = rpool.tile([P, OUT_BATCH * F], f16, tag="res")
                    nc.scalar.activation(
                        res[:, jb * F : (jb + 1) * F],
                        ps[:],
                        mybir.ActivationFunctionType.Copy,
                        scale=inv_sb[:, j : j + 1],
                    )
                    if jb == OUT_BATCH - 1 or j == n_pe - 1:
                        lo = (j - jb) * F
                        nc.scalar.dma_start(
                            out_d[:, lo : (j + 1) * F], res[:, : (jb + 1) * F]
                        )
                    kb += Sw
                t0 += Sb
                target = round((bi + 1) * n_dve / len(batches))
                while vi < min(target, n_dve):
                    do_dve_window(vi)
                    vi += 1
            while vi < n_dve:
                do_dve_window(vi)
                vi += 1

    _split_waits(nc)
    return nc


# ---------------------------------------------------------------------------
def kernel(data, tokens_to_node_map, W=None, b=None, scoring=None):
    global LAST_RESULTS
    from concourse import bass_utils

    in_maps, meta = _preprocess(data, tokens_to_node_map)
    nc = _build_kernel(meta)

    kwargs = {}
    if TRACE and _enable_profiling():
        kwargs["trace"] = True
    res = None
    for attempt in range(3):
        try:
            res = bass_utils.run_bass_kernel_spmd(
                nc, in_maps, core_ids=list(range(N_CORES)), **kwargs
            )
            break
        except Exception:
            if attempt == 2:
                raise
            kwargs.pop("trace", None)  # drop profiling on retry
    LAST_RESULTS = res

    n_win = meta["n_win"]
    wslot = meta["wslot"]
    out = np.zeros((NUM_NODES, F), np.float32)
    for c in range(N_CORES):
        oc = res.results[c]["out"].astype(np.float32)
        oc = oc.reshape(P, n_win, F)
        cn = meta["core_nodes"][c]
        idx = np.flatnonzero(cn >= 0)
        out[cn[idx]] = oc[idx % P, wslot[idx // P]]
    for c in range(N_CORES):
        ids = meta["fb_core"][c]
        if not len(ids):
            continue
        ofb = res.results[c]["outfb"].astype(np.float32)
        for w in range(meta["n_fb_pos"]):
            win = ids[w * P : (w + 1) * P]
            out[win] = ofb[: len(win), w * F : (w + 1) * F]
    return out
